# revision 4
# baseline (speedup 1.0000x reference)
"""Trainium2 Bass kernel for nn_DualDescriptorTS.

Math:  Nk[b,i] = sum_{j,g} x[b,j] * P[i,j,g] * cos(2*pi*k[b]/p[i,j,g]),
       p[i,j,g] = i*1024 + j*16 + g + 2,  x = emb[token_indices].

Sharding: the output i-axis (64) is split across 8 cores (8 i's per core);
each core sees all B=4096 positions, so no cross-core reduction is needed.

Per-core layout: partitions = 128 consecutive period-indices f=(i,j,g)
(fixed i, 8 j's x 16 g's), free axis = all 4096 b.
  1. custom DVE op:  y = a + 1/4 - round(a + 1/4), a = k*invp  (range reduce,
     exact up to f32 rounding; |y| <= 1/2), output z = y
  2. ACT Sin:        phi = sin(2*pi*y) = cos(2*pi*k*invp)
  3. TensorE:        D^T accumulation via zero-padded [128,32] P-weights into
     32-row PSUM regions (4 chunks per region, accumulating matmuls)
  4. DVE+TensorE:    tmp = D * x^T (elementwise), ones-matmul reduces over j.
"""
import numpy as np

import concourse.bacc as bacc
import concourse.tile as tile
from concourse import mybir
from concourse.bass_utils import run_bass_kernel_spmd

# ---------- custom DVE op: centered fractional part ----------
import concourse.dve_ops as dve_ops_mod
from concourse.dve_ops import DveOp
from concourse.dve_spec import Spec, Src0, C0, C1, C2, lower
from concourse.dve_uop import DveOpSpec

_a = Src0 * C0
_u = _a + C2
_t = _u + C1
_m = _t - C1
_FRAC_BODY = _u - _m  # y = (a + 1/4) - round(a + 1/4)  in [-1/2, 1/2]


def _frac_ref(in0, in1, s0, s1, imm2):
    a = in0.astype(np.float32) * np.float32(s0)
    u = (a + np.float32(imm2)).astype(np.float32)
    t = (u + np.float32(s1)).astype(np.float32)
    m = (t - np.float32(s1)).astype(np.float32)
    return (u - m).astype(np.float32)


def _register_frac_op() -> DveOp:
    name = "FRAC_CENTER_ANT"
    for op in dve_ops_mod.OPS:
        if op.name == name:
            return op
    row = dve_ops_mod._CUSTOM_DVE_ROW_BASE + len(dve_ops_mod.OPS)
    assert row < 0x20
    spec = Spec(body=_FRAC_BODY, reference=_frac_ref)
    shas = {}
    for ver in ("v3", "v4"):
        compiled = DveOpSpec(name=name, opcode=row, uops=lower(spec, ver=ver),
                             rd1_en=False)
        shas[ver] = compiled.sha(ver)
    op = DveOp(name, spec, subdim=False, uops_sha=shas)
    dve_ops_mod.OPS.append(op)
    dve_ops_mod.CUSTOM_DVE_SPECS[name] = spec
    dve_ops_mod._SUB_OPCODE_FOR_NAME[name] = row
    return op


FRAC_OP = _register_frac_op()

F32 = mybir.dt.float32
BF16 = mybir.dt.bfloat16
MAGIC = float(np.float32(2.0 ** 23))
TWO_PI = float(2.0 * np.pi)

M, O, B = 64, 16, 4096
NCORES = 8
NCH = 64          # f-chunks of 128 per core
NBB = 8           # b blocks of 512

COL_KREP = 0
COL_XT2 = COL_KREP + B
COL_INVP = COL_XT2 + B
CST_W = COL_INVP + NCH
# bf16 constants tensor layout
COLB_PBLK = 0
COLB_ONES = COLB_PBLK + 32 * NCH
CSTB_W = COLB_ONES + 2

_nc_cache = None
_last_results = None


def _build():
    global _nc_cache
    if _nc_cache is not None:
        return _nc_cache
    nc = bacc.Bacc(target_bir_lowering=False, debug=False)
    cst_d = nc.declare_dram_parameter("cst", [128, CST_W], F32, isOutput=False)
    cstb_d = nc.declare_dram_parameter("cstb", [128, CSTB_W], BF16, isOutput=False)
    out_d = nc.declare_dram_parameter("out", [8, B], F32, isOutput=True)

    with tile.TileContext(nc) as tc:
        with (
            tc.tile_pool(name="cstp", bufs=1) as cpool,
            tc.tile_pool(name="zp", bufs=3) as zpool,
            tc.tile_pool(name="php", bufs=3) as ppool,
            tc.tile_pool(name="tmp", bufs=4) as tpool,
            tc.tile_pool(name="nkp", bufs=2) as npool,
            tc.tile_pool(name="gup", bufs=1) as gupool,
            tc.tile_pool(name="gmp", bufs=1) as gmpool,
            tc.tile_pool(name="ps", bufs=8, space="PSUM") as psum,
        ):
            cst = cpool.tile([128, CST_W], F32)
            nc.sync.dma_start(cst[:], cst_d[:])
            cstb = cpool.tile([128, CSTB_W], BF16)
            nc.sync.dma_start(cstb[:], cstb_d[:])
            krep = cst[:, COL_KREP:COL_KREP + B]

            for ip in range(4):
                dps = [psum.tile([128, 512], F32, tag="dps", name=f"dps{ip}_{b_}") for b_ in range(NBB)]
                nk_t = npool.tile([2, B], F32)
                for cc in range(16):
                    ch = ip * 16 + cc
                    z = zpool.tile([128, B], F32)
                    if ch % 5 == 2:
                        # offload this chunk's range reduction to GpSimd
                        u = gupool.tile([128, B], F32)
                        nc.gpsimd.tensor_scalar(
                            u[:], krep,
                            cst[:, COL_INVP + ch:COL_INVP + ch + 1], 0.25,
                            mybir.AluOpType.mult, mybir.AluOpType.add)
                        m = gmpool.tile([128, B], F32)
                        nc.gpsimd.tensor_scalar(
                            m[:], u[:], MAGIC, MAGIC,
                            mybir.AluOpType.add, mybir.AluOpType.subtract)
                        nc.gpsimd.tensor_tensor(z[:], u[:], m[:],
                                                mybir.AluOpType.subtract)
                    else:
                        nc.vector._custom_dve(
                            FRAC_OP, out=z[:], in0=krep,
                            s0=cst[:, COL_INVP + ch:COL_INVP + ch + 1],
                            s1=MAGIC, imm2=0.25)
                    phi = ppool.tile([128, B], BF16)
                    nc.scalar.activation(phi[:], z[:],
                                         mybir.ActivationFunctionType.Sin,
                                         bias=0.0, scale=TWO_PI)
                    grp, slot = cc // 4, cc % 4
                    for bb in range(NBB):
                        nc.tensor.matmul(
                            dps[bb][32 * grp:32 * grp + 32, :],
                            cstb[:, COLB_PBLK + 32 * ch:COLB_PBLK + 32 * ch + 32],
                            phi[:, 512 * bb:512 * bb + 512],
                            start=(slot == 0), stop=(slot == 3),
                            tile_position=(0, 32 * grp))
                for bb in range(NBB):
                    tmp = tpool.tile([128, 512], BF16)
                    nc.vector.tensor_tensor(
                        tmp[:], dps[bb][:, :],
                        cst[:, COL_XT2 + 512 * bb:COL_XT2 + 512 * bb + 512],
                        mybir.AluOpType.mult)
                    nc.tensor.matmul(dps[bb][0:2, :],
                                     cstb[:, COLB_ONES:COLB_ONES + 2], tmp[:],
                                     start=True, stop=True)
                    nc.scalar.copy(nk_t[:, 512 * bb:512 * bb + 512],
                                   dps[bb][0:2, :])
                nc.sync.dma_start(out_d[2 * ip:2 * ip + 2, :], nk_t[:])
    nc.compile()
    _nc_cache = nc
    return nc


def _make_inputs(k_tensor, token_indices, emb, P):
    k = np.asarray(k_tensor, dtype=np.float32).reshape(B)
    tok = np.asarray(token_indices).astype(np.int64).reshape(B)
    emb_ = np.asarray(emb, dtype=np.float32)
    P_ = np.asarray(P, dtype=np.float32)

    x = emb_[tok]                                    # [B, 64]
    xt2 = np.concatenate([x.T, x.T], axis=0)         # [128, B]
    krep = np.broadcast_to(k, (128, B))
    invp_all = (1.0 / (np.arange(M * M * O, dtype=np.float64) + 2.0)
                ).astype(np.float32)

    import ml_dtypes
    bf16 = ml_dtypes.bfloat16
    in_maps = []
    for c in range(NCORES):
        cst = np.zeros((128, CST_W), dtype=np.float32)
        cst[:, COL_KREP:COL_KREP + B] = krep
        cst[:, COL_XT2:COL_XT2 + B] = xt2
        base = NCH * c
        cst[:, COL_INVP:COL_INVP + NCH] = (
            invp_all[128 * base:128 * (base + NCH)].reshape(NCH, 128).T)
        cstb = np.zeros((128, CSTB_W), dtype=np.float32)
        for ch in range(NCH):
            g = base + ch
            i, sub = g // 8, g % 8
            col0 = COLB_PBLK + 32 * ch + 8 * (ch % 4)
            for jl in range(8):
                cstb[16 * jl:16 * jl + 16, col0 + jl] = P_[i, 8 * sub + jl, :]
        cstb[0:64, COLB_ONES] = 1.0
        cstb[64:128, COLB_ONES + 1] = 1.0
        in_maps.append({"cst": cst, "cstb": cstb.astype(bf16)})
    return in_maps


def kernel(k_tensor, token_indices, emb, P):
    global _last_results
    nc = _build()
    in_maps = _make_inputs(k_tensor, token_indices, emb, P)
    res = run_bass_kernel_spmd(nc, in_maps, list(range(NCORES)))
    _last_results = res
    out = np.concatenate([res.results[c]["out"] for c in range(NCORES)],
                         axis=0)                     # [64, B] (i, b)
    return np.ascontiguousarray(out.T).astype(np.float32)   # [B, 64]


# revision 5
# speedup vs baseline: 2.8334x; 2.8334x over previous
"""Trainium2 Bass kernel for nn_DualDescriptorTS.

Math:  Nk[b,i] = sum_{j,g} x[b,j] * P[i,j,g] * cos(2*pi*k[b]/p[i,j,g]),
       p[i,j,g] = i*1024 + j*16 + g + 2,  x = emb[token_indices].

Sharding: the output i-axis (64) is split across 8 cores (8 i's per core);
each core sees all B=4096 positions, so no cross-core reduction is needed.

Per-core layout: partitions = 128 consecutive period-indices f=(i,j,g)
(fixed i, 8 j's x 16 g's), free axis = all 4096 b.
  1. custom DVE op:  y = a + 1/4 - round(a + 1/4), a = k*invp  (range reduce,
     exact up to f32 rounding; |y| <= 1/2), output z = y
  2. ACT Sin:        phi = sin(2*pi*y) = cos(2*pi*k*invp)
  3. TensorE:        D^T accumulation via zero-padded [128,32] P-weights into
     32-row PSUM regions (4 chunks per region, accumulating matmuls)
  4. DVE+TensorE:    tmp = D * x^T (elementwise), ones-matmul reduces over j.
"""
import numpy as np

import concourse.bacc as bacc
import concourse.tile as tile
from concourse import mybir
from concourse.bass_utils import run_bass_kernel_spmd

# ---------- custom DVE op: centered fractional part ----------
import concourse.dve_ops as dve_ops_mod
from concourse.dve_ops import DveOp
from concourse.dve_spec import Spec, Src0, C0, C1, C2, lower
from concourse.dve_uop import DveOpSpec

_a = Src0 * C0
_u = _a + C2
_t = _u + C1
_m = _t - C1
_FRAC_BODY = _u - _m  # y = (a + 1/4) - round(a + 1/4)  in [-1/2, 1/2]


def _frac_ref(in0, in1, s0, s1, imm2):
    a = in0.astype(np.float32) * np.float32(s0)
    u = (a + np.float32(imm2)).astype(np.float32)
    t = (u + np.float32(s1)).astype(np.float32)
    m = (t - np.float32(s1)).astype(np.float32)
    return (u - m).astype(np.float32)


def _register_frac_op() -> DveOp:
    name = "FRAC_CENTER_ANT"
    for op in dve_ops_mod.OPS:
        if op.name == name:
            return op
    row = dve_ops_mod._CUSTOM_DVE_ROW_BASE + len(dve_ops_mod.OPS)
    assert row < 0x20
    spec = Spec(body=_FRAC_BODY, reference=_frac_ref)
    shas = {}
    for ver in ("v3", "v4"):
        compiled = DveOpSpec(name=name, opcode=row, uops=lower(spec, ver=ver),
                             rd1_en=False)
        shas[ver] = compiled.sha(ver)
    op = DveOp(name, spec, subdim=False, uops_sha=shas)
    dve_ops_mod.OPS.append(op)
    dve_ops_mod.CUSTOM_DVE_SPECS[name] = spec
    dve_ops_mod._SUB_OPCODE_FOR_NAME[name] = row
    return op


FRAC_OP = _register_frac_op()

F32 = mybir.dt.float32
BF16 = mybir.dt.bfloat16
MAGIC = float(np.float32(2.0 ** 23))
TWO_PI = float(2.0 * np.pi)

M, O, B = 64, 16, 4096
NCORES = 8
NCH = 64          # f-chunks of 128 per core
NBB = 8           # b blocks of 512

COL_KREP = 0
COL_XT2 = COL_KREP + B
COL_INVP = COL_XT2 + B
CST_W = COL_INVP + NCH
# bf16 constants tensor layout
COLB_PBLK = 0
COLB_ONES = COLB_PBLK + 32 * NCH
CSTB_W = COLB_ONES + 2

_nc_cache = None
_last_results = None


def _build():
    global _nc_cache
    if _nc_cache is not None:
        return _nc_cache
    nc = bacc.Bacc(target_bir_lowering=False, debug=False)
    cst_d = nc.declare_dram_parameter("cst", [128, CST_W], F32, isOutput=False)
    cstb_d = nc.declare_dram_parameter("cstb", [128, CSTB_W], BF16, isOutput=False)
    out_d = nc.declare_dram_parameter("out", [8, B], F32, isOutput=True)

    with tile.TileContext(nc) as tc:
        with (
            tc.tile_pool(name="cstp", bufs=1) as cpool,
            tc.tile_pool(name="zp", bufs=3) as zpool,
            tc.tile_pool(name="php", bufs=3) as ppool,
            tc.tile_pool(name="tmp", bufs=4) as tpool,
            tc.tile_pool(name="nkp", bufs=2) as npool,
            tc.tile_pool(name="ps", bufs=8, space="PSUM") as psum,
        ):
            cst = cpool.tile([128, CST_W], F32)
            nc.sync.dma_start(cst[:], cst_d[:])
            cstb = cpool.tile([128, CSTB_W], BF16)
            nc.sync.dma_start(cstb[:], cstb_d[:])
            krep = cst[:, COL_KREP:COL_KREP + B]
            neghalfpi = cpool.tile([128, 1], F32)
            nc.vector.memset(neghalfpi[:], -float(np.pi) / 2.0)

            for ip in range(4):
                dps = [psum.tile([128, 512], F32, tag="dps", name=f"dps{ip}_{b_}") for b_ in range(NBB)]
                nk_t = npool.tile([2, B], F32)
                for cc in range(16):
                    ch = ip * 16 + cc
                    phi = ppool.tile([128, B], BF16)
                    if ch < 8:
                        # small periods (p < 8194): explicit range reduction on
                        # DVE: z = (a - 1/4) - round(a - 1/4), a = k*invp;
                        # sin(2*pi*z) = -cos(2*pi*a)
                        z = zpool.tile([128, B], F32)
                        nc.vector._custom_dve(
                            FRAC_OP, out=z[:], in0=krep,
                            s0=cst[:, COL_INVP + ch:COL_INVP + ch + 1],
                            s1=MAGIC, imm2=-0.25)
                        nc.scalar.activation(phi[:], z[:],
                                             mybir.ActivationFunctionType.Sin,
                                             bias=0.0, scale=TWO_PI)
                    else:
                        # large periods: 2*pi*k*invp < pi, so
                        # sin(2*pi*invp*k - pi/2) = -cos(2*pi*k*invp) directly;
                        # scale column holds 2*pi*invp
                        nc.scalar.activation(
                            phi[:], krep,
                            mybir.ActivationFunctionType.Sin,
                            bias=neghalfpi[:, 0:1],
                            scale=cst[:, COL_INVP + ch:COL_INVP + ch + 1])
                    grp, slot = cc // 4, cc % 4
                    for bb in range(NBB):
                        nc.tensor.matmul(
                            dps[bb][32 * grp:32 * grp + 32, :],
                            cstb[:, COLB_PBLK + 32 * ch:COLB_PBLK + 32 * ch + 32],
                            phi[:, 512 * bb:512 * bb + 512],
                            start=(slot == 0), stop=(slot == 3),
                            tile_position=(0, 32 * grp))
                for bb in range(NBB):
                    tmp = tpool.tile([128, 512], BF16)
                    nc.vector.tensor_tensor(
                        tmp[:], dps[bb][:, :],
                        cst[:, COL_XT2 + 512 * bb:COL_XT2 + 512 * bb + 512],
                        mybir.AluOpType.mult)
                    nc.tensor.matmul(dps[bb][0:2, :],
                                     cstb[:, COLB_ONES:COLB_ONES + 2], tmp[:],
                                     start=True, stop=True)
                    nc.vector.tensor_copy(nk_t[:, 512 * bb:512 * bb + 512],
                                           dps[bb][0:2, :])
                nc.sync.dma_start(out_d[2 * ip:2 * ip + 2, :], nk_t[:])
    nc.compile()
    _nc_cache = nc
    return nc


def _make_inputs(k_tensor, token_indices, emb, P):
    k = np.asarray(k_tensor, dtype=np.float32).reshape(B)
    tok = np.asarray(token_indices).astype(np.int64).reshape(B)
    emb_ = np.asarray(emb, dtype=np.float32)
    P_ = np.asarray(P, dtype=np.float32)

    x = emb_[tok]                                    # [B, 64]
    xt2 = np.concatenate([x.T, x.T], axis=0)         # [128, B]
    krep = np.broadcast_to(k, (128, B))
    invp_all = (1.0 / (np.arange(M * M * O, dtype=np.float64) + 2.0)
                ).astype(np.float32)

    import ml_dtypes
    bf16 = ml_dtypes.bfloat16
    in_maps = []
    for c in range(NCORES):
        cst = np.zeros((128, CST_W), dtype=np.float32)
        cst[:, COL_KREP:COL_KREP + B] = krep
        cst[:, COL_XT2:COL_XT2 + B] = xt2
        cstb = np.zeros((128, CSTB_W), dtype=np.float32)
        for ch in range(NCH):
            i = c + 8 * (ch // 8)
            sub = ch % 8
            g = 8 * i + sub
            col = invp_all[128 * g:128 * (g + 1)]
            if ch >= 8:
                col = (2.0 * np.pi * col.astype(np.float64)).astype(np.float32)
            cst[:, COL_INVP + ch] = col
            col0 = COLB_PBLK + 32 * ch + 8 * (ch % 4)
            for jl in range(8):
                cstb[16 * jl:16 * jl + 16, col0 + jl] = -P_[i, 8 * sub + jl, :]
        cstb[0:64, COLB_ONES] = 1.0
        cstb[64:128, COLB_ONES + 1] = 1.0
        in_maps.append({"cst": cst, "cstb": cstb.astype(bf16)})
    return in_maps


def kernel(k_tensor, token_indices, emb, P):
    global _last_results
    nc = _build()
    in_maps = _make_inputs(k_tensor, token_indices, emb, P)
    res = run_bass_kernel_spmd(nc, in_maps, list(range(NCORES)))
    _last_results = res
    out = np.empty((M, B), dtype=np.float32)         # [i, b]
    for c in range(NCORES):
        out[c::8] = res.results[c]["out"]            # rows r -> i = c + 8*r
    return np.ascontiguousarray(out.T).astype(np.float32)   # [B, 64]


# revision 6
# speedup vs baseline: 3.2551x; 1.1488x over previous
"""Trainium2 Bass kernel for nn_DualDescriptorTS.

Math:  Nk[b,i] = sum_{j,g} x[b,j] * P[i,j,g] * cos(2*pi*k[b]/p[i,j,g]),
       p[i,j,g] = i*1024 + j*16 + g + 2,  x = emb[token_indices].

Sharding: the output i-axis (64) is split across 8 cores (8 i's per core);
each core sees all B=4096 positions, so no cross-core reduction is needed.

Per-core layout: partitions = 128 consecutive period-indices f=(i,j,g)
(fixed i, 8 j's x 16 g's), free axis = all 4096 b.
  1. custom DVE op:  y = a + 1/4 - round(a + 1/4), a = k*invp  (range reduce,
     exact up to f32 rounding; |y| <= 1/2), output z = y
  2. ACT Sin:        phi = sin(2*pi*y) = cos(2*pi*k*invp)
  3. TensorE:        D^T accumulation via zero-padded [128,32] P-weights into
     32-row PSUM regions (4 chunks per region, accumulating matmuls)
  4. DVE+TensorE:    tmp = D * x^T (elementwise), ones-matmul reduces over j.
"""
import numpy as np

import concourse.bacc as bacc
import concourse.tile as tile
from concourse import mybir
from concourse.bass_utils import run_bass_kernel_spmd

# ---------- custom DVE op: centered fractional part ----------
import concourse.dve_ops as dve_ops_mod
from concourse.dve_ops import DveOp
from concourse.dve_spec import Spec, Src0, C0, C1, C2, lower
from concourse.dve_uop import DveOpSpec

_a = Src0 * C0
_u = _a + C2
_t = _u + C1
_m = _t - C1
_FRAC_BODY = _u - _m  # y = (a + 1/4) - round(a + 1/4)  in [-1/2, 1/2]


def _frac_ref(in0, in1, s0, s1, imm2):
    a = in0.astype(np.float32) * np.float32(s0)
    u = (a + np.float32(imm2)).astype(np.float32)
    t = (u + np.float32(s1)).astype(np.float32)
    m = (t - np.float32(s1)).astype(np.float32)
    return (u - m).astype(np.float32)


def _register_frac_op() -> DveOp:
    name = "FRAC_CENTER_ANT"
    for op in dve_ops_mod.OPS:
        if op.name == name:
            return op
    row = dve_ops_mod._CUSTOM_DVE_ROW_BASE + len(dve_ops_mod.OPS)
    assert row < 0x20
    spec = Spec(body=_FRAC_BODY, reference=_frac_ref)
    shas = {}
    for ver in ("v3", "v4"):
        compiled = DveOpSpec(name=name, opcode=row, uops=lower(spec, ver=ver),
                             rd1_en=False)
        shas[ver] = compiled.sha(ver)
    op = DveOp(name, spec, subdim=False, uops_sha=shas)
    dve_ops_mod.OPS.append(op)
    dve_ops_mod.CUSTOM_DVE_SPECS[name] = spec
    dve_ops_mod._SUB_OPCODE_FOR_NAME[name] = row
    return op


FRAC_OP = _register_frac_op()

F32 = mybir.dt.float32
BF16 = mybir.dt.bfloat16
MAGIC = float(np.float32(2.0 ** 23))
TWO_PI = float(2.0 * np.pi)

M, O, B = 64, 16, 4096
NCORES = 8
NCH = 64          # f-chunks of 128 per core
NBB = 8           # b blocks of 512

COL_KREP = 0
COL_XT2 = COL_KREP + B
COL_INVP = COL_XT2 + B
CST_W = COL_INVP + NCH
# bf16 constants tensor layout
COLB_PBLK = 0
COLB_ONES = COLB_PBLK + 32 * NCH
CSTB_W = COLB_ONES + 2

_nc_cache = None
_last_results = None


def _build():
    global _nc_cache
    if _nc_cache is not None:
        return _nc_cache
    nc = bacc.Bacc(target_bir_lowering=False, debug=False)
    cst_d = nc.declare_dram_parameter("cst", [128, CST_W], F32, isOutput=False)
    cstb_d = nc.declare_dram_parameter("cstb", [128, CSTB_W], BF16, isOutput=False)
    out_d = nc.declare_dram_parameter("out", [8, B], F32, isOutput=True)

    with tile.TileContext(nc) as tc:
        with (
            tc.tile_pool(name="cstp", bufs=1) as cpool,
            tc.tile_pool(name="zp", bufs=3) as zpool,
            tc.tile_pool(name="php", bufs=3) as ppool,
            tc.tile_pool(name="tmp", bufs=4) as tpool,
            tc.tile_pool(name="nkp", bufs=2) as npool,
            tc.tile_pool(name="ps", bufs=8, space="PSUM") as psum,
        ):
            cst = cpool.tile([128, CST_W], F32)
            nc.sync.dma_start(cst[:], cst_d[:])
            cstb = cpool.tile([128, CSTB_W], BF16)
            nc.sync.dma_start(cstb[:], cstb_d[:])
            krep = cst[:, COL_KREP:COL_KREP + B]

            for ip in range(4):
                dps = [psum.tile([128, 512], F32, tag="dps", name=f"dps{ip}_{b_}") for b_ in range(NBB)]
                nk_t = npool.tile([2, B], F32)
                for cc in range(16):
                    ch = ip * 16 + cc
                    phi = ppool.tile([128, B], BF16)
                    if ch < 8:
                        # small periods (p < 8194): explicit range reduction on
                        # DVE: z = (a - 1/4) - round(a - 1/4), a = k*invp;
                        # sin(2*pi*z) = -cos(2*pi*a)
                        z = zpool.tile([128, B], F32, name=f"z{ch}", tag="zw")
                        nc.vector._custom_dve(
                            FRAC_OP, out=z[:], in0=krep,
                            s0=cst[:, COL_INVP + ch:COL_INVP + ch + 1],
                            s1=MAGIC, imm2=-0.25)
                        nc.scalar.activation(phi[:], z[:],
                                             mybir.ActivationFunctionType.Sin,
                                             bias=0.0, scale=TWO_PI)
                    else:
                        # large periods: 2*pi*k*invp < pi, so
                        # sin(2*pi*invp*k - pi/2) = -cos(2*pi*k*invp); compute
                        # w = k*(2*pi*invp) - pi/2 on DVE (2x mode), sin on ACT
                        # with immediate scale/bias (faster than AP operands)
                        w = zpool.tile([128, B], F32, name=f"w{ch}", tag="zw")
                        nc.vector.tensor_scalar(
                            w[:], krep,
                            cst[:, COL_INVP + ch:COL_INVP + ch + 1],
                            -float(np.pi) / 2.0,
                            mybir.AluOpType.mult, mybir.AluOpType.add)
                        nc.scalar.activation(phi[:], w[:],
                                             mybir.ActivationFunctionType.Sin,
                                             bias=0.0, scale=1.0)
                    grp, slot = cc // 4, cc % 4
                    for bb in range(NBB):
                        nc.tensor.matmul(
                            dps[bb][32 * grp:32 * grp + 32, :],
                            cstb[:, COLB_PBLK + 32 * ch:COLB_PBLK + 32 * ch + 32],
                            phi[:, 512 * bb:512 * bb + 512],
                            start=(slot == 0), stop=(slot == 3),
                            tile_position=(0, 32 * grp))
                for bb in range(NBB):
                    tmp = tpool.tile([128, 512], BF16)
                    nc.vector.tensor_tensor(
                        tmp[:], dps[bb][:, :],
                        cst[:, COL_XT2 + 512 * bb:COL_XT2 + 512 * bb + 512],
                        mybir.AluOpType.mult)
                    nc.tensor.matmul(dps[bb][0:2, :],
                                     cstb[:, COLB_ONES:COLB_ONES + 2], tmp[:],
                                     start=True, stop=True)
                    nc.vector.tensor_copy(nk_t[:, 512 * bb:512 * bb + 512],
                                           dps[bb][0:2, :])
                nc.sync.dma_start(out_d[2 * ip:2 * ip + 2, :], nk_t[:])
    nc.compile()
    _nc_cache = nc
    return nc


def _make_inputs(k_tensor, token_indices, emb, P):
    k = np.asarray(k_tensor, dtype=np.float32).reshape(B)
    tok = np.asarray(token_indices).astype(np.int64).reshape(B)
    emb_ = np.asarray(emb, dtype=np.float32)
    P_ = np.asarray(P, dtype=np.float32)

    x = emb_[tok]                                    # [B, 64]
    xt2 = np.concatenate([x.T, x.T], axis=0)         # [128, B]
    krep = np.broadcast_to(k, (128, B))
    invp_all = (1.0 / (np.arange(M * M * O, dtype=np.float64) + 2.0)
                ).astype(np.float32)

    import ml_dtypes
    bf16 = ml_dtypes.bfloat16
    in_maps = []
    for c in range(NCORES):
        cst = np.zeros((128, CST_W), dtype=np.float32)
        cst[:, COL_KREP:COL_KREP + B] = krep
        cst[:, COL_XT2:COL_XT2 + B] = xt2
        cstb = np.zeros((128, CSTB_W), dtype=np.float32)
        for ch in range(NCH):
            i = c + 8 * (ch // 8)
            sub = ch % 8
            g = 8 * i + sub
            col = invp_all[128 * g:128 * (g + 1)]
            if ch >= 8:
                col = (2.0 * np.pi * col.astype(np.float64)).astype(np.float32)
            cst[:, COL_INVP + ch] = col
            col0 = COLB_PBLK + 32 * ch + 8 * (ch % 4)
            for jl in range(8):
                cstb[16 * jl:16 * jl + 16, col0 + jl] = -P_[i, 8 * sub + jl, :]
        cstb[0:64, COLB_ONES] = 1.0
        cstb[64:128, COLB_ONES + 1] = 1.0
        in_maps.append({"cst": cst, "cstb": cstb.astype(bf16)})
    return in_maps


def kernel(k_tensor, token_indices, emb, P):
    global _last_results
    nc = _build()
    in_maps = _make_inputs(k_tensor, token_indices, emb, P)
    res = run_bass_kernel_spmd(nc, in_maps, list(range(NCORES)))
    _last_results = res
    out = np.empty((M, B), dtype=np.float32)         # [i, b]
    for c in range(NCORES):
        out[c::8] = res.results[c]["out"]            # rows r -> i = c + 8*r
    return np.ascontiguousarray(out.T).astype(np.float32)   # [B, 64]


# revision 7
# speedup vs baseline: 3.4984x; 1.0747x over previous
"""Trainium2 Bass kernel for nn_DualDescriptorTS.

Math:  Nk[b,i] = sum_{j,g} x[b,j] * P[i,j,g] * cos(2*pi*k[b]/p[i,j,g]),
       p[i,j,g] = i*1024 + j*16 + g + 2,  x = emb[token_indices].

Sharding: the output i-axis (64) is split across 8 cores (8 i's per core);
each core sees all B=4096 positions, so no cross-core reduction is needed.

Per-core layout: partitions = 128 consecutive period-indices f=(i,j,g)
(fixed i, 8 j's x 16 g's), free axis = all 4096 b.
  1. custom DVE op:  y = a + 1/4 - round(a + 1/4), a = k*invp  (range reduce,
     exact up to f32 rounding; |y| <= 1/2), output z = y
  2. ACT Sin:        phi = sin(2*pi*y) = cos(2*pi*k*invp)
  3. TensorE:        D^T accumulation via zero-padded [128,32] P-weights into
     32-row PSUM regions (4 chunks per region, accumulating matmuls)
  4. DVE+TensorE:    tmp = D * x^T (elementwise), ones-matmul reduces over j.
"""
import numpy as np

import concourse.bacc as bacc
import concourse.tile as tile
from concourse import mybir
from concourse.bass_utils import run_bass_kernel_spmd

# ---------- custom DVE op: centered fractional part ----------
import concourse.dve_ops as dve_ops_mod
from concourse.dve_ops import DveOp
from concourse.dve_spec import Spec, Src0, C0, C1, C2, lower
from concourse.dve_uop import DveOpSpec

_a = Src0 * C0
_u = _a + C2
_t = _u + C1
_m = _t - C1
_FRAC_BODY = _u - _m  # y = (a + 1/4) - round(a + 1/4)  in [-1/2, 1/2]


def _frac_ref(in0, in1, s0, s1, imm2):
    a = in0.astype(np.float32) * np.float32(s0)
    u = (a + np.float32(imm2)).astype(np.float32)
    t = (u + np.float32(s1)).astype(np.float32)
    m = (t - np.float32(s1)).astype(np.float32)
    return (u - m).astype(np.float32)


def _register_frac_op() -> DveOp:
    name = "FRAC_CENTER_ANT"
    for op in dve_ops_mod.OPS:
        if op.name == name:
            return op
    row = dve_ops_mod._CUSTOM_DVE_ROW_BASE + len(dve_ops_mod.OPS)
    assert row < 0x20
    spec = Spec(body=_FRAC_BODY, reference=_frac_ref)
    shas = {}
    for ver in ("v3", "v4"):
        compiled = DveOpSpec(name=name, opcode=row, uops=lower(spec, ver=ver),
                             rd1_en=False)
        shas[ver] = compiled.sha(ver)
    op = DveOp(name, spec, subdim=False, uops_sha=shas)
    dve_ops_mod.OPS.append(op)
    dve_ops_mod.CUSTOM_DVE_SPECS[name] = spec
    dve_ops_mod._SUB_OPCODE_FOR_NAME[name] = row
    return op


FRAC_OP = _register_frac_op()

F32 = mybir.dt.float32
BF16 = mybir.dt.bfloat16
MAGIC = float(np.float32(2.0 ** 23))
TWO_PI = float(2.0 * np.pi)

M, O, B = 64, 16, 4096
NCORES = 8
NCH = 64          # f-chunks of 128 per core
NBB = 8           # b blocks of 512

COL_KREP = 0
COL_XT2 = COL_KREP + B
COL_INVP = COL_XT2 + B
CST_W = COL_INVP + NCH
# bf16 constants tensor layout
COLB_PBLK = 0
COLB_ONES = COLB_PBLK + 32 * NCH
CSTB_W = COLB_ONES + 2

_nc_cache = None
_last_results = None


def _build():
    global _nc_cache
    if _nc_cache is not None:
        return _nc_cache
    nc = bacc.Bacc(target_bir_lowering=False, debug=False)
    cst_d = nc.declare_dram_parameter("cst", [128, CST_W], F32, isOutput=False)
    cstb_d = nc.declare_dram_parameter("cstb", [128, CSTB_W], BF16, isOutput=False)
    out_d = nc.declare_dram_parameter("out", [8, B], F32, isOutput=True)

    with tile.TileContext(nc) as tc:
        with (
            tc.tile_pool(name="cstp", bufs=1) as cpool,
            tc.tile_pool(name="zp", bufs=4) as zpool,
            tc.tile_pool(name="php", bufs=5) as ppool,
            tc.tile_pool(name="tmp", bufs=6) as tpool,
            tc.tile_pool(name="nkp", bufs=2) as npool,
            tc.tile_pool(name="ps", bufs=8, space="PSUM") as psum,
        ):
            cst = cpool.tile([128, CST_W], F32)
            # split the constant load: krep+invp (needed first), then xt2
            nc.sync.dma_start(cst[:, COL_KREP:COL_KREP + B],
                              cst_d[:, COL_KREP:COL_KREP + B])
            nc.sync.dma_start(cst[:, COL_INVP:COL_INVP + NCH],
                              cst_d[:, COL_INVP:COL_INVP + NCH])
            cstb = cpool.tile([128, CSTB_W], BF16)
            nc.sync.dma_start(cstb[:], cstb_d[:])
            nc.sync.dma_start(cst[:, COL_XT2:COL_XT2 + B],
                              cst_d[:, COL_XT2:COL_XT2 + B])
            krep = cst[:, COL_KREP:COL_KREP + B]

            for ip in range(4):
                dps = [psum.tile([128, 512], F32, tag="dps", name=f"dps{ip}_{b_}") for b_ in range(NBB)]
                nk_t = npool.tile([2, B], F32)
                for cc in range(16):
                    ch = ip * 16 + cc
                    phi = ppool.tile([128, B], BF16)
                    if ch < 8:
                        # small periods (p < 8194): explicit range reduction on
                        # DVE: z = (a - 1/4) - round(a - 1/4), a = k*invp;
                        # sin(2*pi*z) = -cos(2*pi*a)
                        z = zpool.tile([128, B], F32, name=f"z{ch}", tag="zw")
                        nc.vector._custom_dve(
                            FRAC_OP, out=z[:], in0=krep,
                            s0=cst[:, COL_INVP + ch:COL_INVP + ch + 1],
                            s1=MAGIC, imm2=-0.25)
                        nc.scalar.activation(phi[:], z[:],
                                             mybir.ActivationFunctionType.Sin,
                                             bias=0.0, scale=TWO_PI)
                    else:
                        # large periods: 2*pi*k*invp < pi, so
                        # sin(2*pi*invp*k - pi/2) = -cos(2*pi*k*invp); compute
                        # w = k*(2*pi*invp) - pi/2 on DVE (2x mode), sin on ACT
                        # with immediate scale/bias (faster than AP operands)
                        w = zpool.tile([128, B], F32, name=f"w{ch}", tag="zw")
                        nc.vector.tensor_scalar(
                            w[:], krep,
                            cst[:, COL_INVP + ch:COL_INVP + ch + 1],
                            -float(np.pi) / 2.0,
                            mybir.AluOpType.mult, mybir.AluOpType.add)
                        nc.scalar.activation(phi[:], w[:],
                                             mybir.ActivationFunctionType.Sin,
                                             bias=0.0, scale=1.0)
                    grp, slot = cc // 4, cc % 4
                    for bb in range(NBB):
                        nc.tensor.matmul(
                            dps[bb][32 * grp:32 * grp + 32, :],
                            cstb[:, COLB_PBLK + 32 * ch:COLB_PBLK + 32 * ch + 32],
                            phi[:, 512 * bb:512 * bb + 512],
                            start=(slot == 0), stop=(slot == 3),
                            tile_position=(0, 32 * grp))
                for bb in range(NBB):
                    tmp = tpool.tile([128, 512], BF16)
                    nc.vector.tensor_tensor(
                        tmp[:], dps[bb][:, :],
                        cst[:, COL_XT2 + 512 * bb:COL_XT2 + 512 * bb + 512],
                        mybir.AluOpType.mult)
                    nc.tensor.matmul(dps[bb][0:2, :],
                                     cstb[:, COLB_ONES:COLB_ONES + 2], tmp[:],
                                     start=True, stop=True)
                    nc.vector.tensor_copy(nk_t[:, 512 * bb:512 * bb + 512],
                                           dps[bb][0:2, :])
                nc.sync.dma_start(out_d[2 * ip:2 * ip + 2, :], nk_t[:])
    nc.compile()
    _nc_cache = nc
    return nc


def _make_inputs(k_tensor, token_indices, emb, P):
    k = np.asarray(k_tensor, dtype=np.float32).reshape(B)
    tok = np.asarray(token_indices).astype(np.int64).reshape(B)
    emb_ = np.asarray(emb, dtype=np.float32)
    P_ = np.asarray(P, dtype=np.float32)

    x = emb_[tok]                                    # [B, 64]
    xt2 = np.concatenate([x.T, x.T], axis=0)         # [128, B]
    krep = np.broadcast_to(k, (128, B))
    invp_all = (1.0 / (np.arange(M * M * O, dtype=np.float64) + 2.0)
                ).astype(np.float32)

    import ml_dtypes
    bf16 = ml_dtypes.bfloat16
    in_maps = []
    for c in range(NCORES):
        cst = np.zeros((128, CST_W), dtype=np.float32)
        cst[:, COL_KREP:COL_KREP + B] = krep
        cst[:, COL_XT2:COL_XT2 + B] = xt2
        cstb = np.zeros((128, CSTB_W), dtype=np.float32)
        for ch in range(NCH):
            i = c + 8 * (ch // 8)
            sub = ch % 8
            g = 8 * i + sub
            col = invp_all[128 * g:128 * (g + 1)]
            if ch >= 8:
                col = (2.0 * np.pi * col.astype(np.float64)).astype(np.float32)
            cst[:, COL_INVP + ch] = col
            col0 = COLB_PBLK + 32 * ch + 8 * (ch % 4)
            for jl in range(8):
                cstb[16 * jl:16 * jl + 16, col0 + jl] = -P_[i, 8 * sub + jl, :]
        cstb[0:64, COLB_ONES] = 1.0
        cstb[64:128, COLB_ONES + 1] = 1.0
        in_maps.append({"cst": cst, "cstb": cstb.astype(bf16)})
    return in_maps


def kernel(k_tensor, token_indices, emb, P):
    global _last_results
    nc = _build()
    in_maps = _make_inputs(k_tensor, token_indices, emb, P)
    res = run_bass_kernel_spmd(nc, in_maps, list(range(NCORES)))
    _last_results = res
    out = np.empty((M, B), dtype=np.float32)         # [i, b]
    for c in range(NCORES):
        out[c::8] = res.results[c]["out"]            # rows r -> i = c + 8*r
    return np.ascontiguousarray(out.T).astype(np.float32)   # [B, 64]
